# revision 1
# baseline (speedup 1.0000x reference)
import numpy as np
import jax
import jax.numpy as jnp
from functools import partial

# nn_LstmDecoder: V=1000, E=H=512, L=2, S=64, B=512, T=64
V, E, H, L = 1000, 512, 512, 2
S, B, T = 64, 512, 64
NEG = -9999.9
RATIOS = (1.0, 0.2)
NCORES = 8
BL = B // NCORES  # 64 batch per core


def _nc_residual(inputs, ratios):
    running = None
    out = 0.0
    for x, r in zip(inputs, ratios):
        n = jnp.linalg.norm(x, axis=-1, keepdims=True) + 1e-8
        tgt = n * r if running is None else running * r
        out = out + x / n * tgt
        running = tgt
    return out


def _decode_local(src_emb, src_outputs, mask_src, target, x_all,
                  w_ih, w_hh, b_ih, b_hh, Wa, W_hid, b_hid, emb_table):
    # src_emb/src_outputs: [S, BL, E]; mask: [BL, S]; target: [T, BL]
    # x_all: [T, BL, E] pre-gathered embeddings of the input tokens
    Bn = src_emb.shape[1]
    Wh_s = jnp.einsum('sbd,dh->sbh', src_outputs, Wa)
    h0 = jnp.zeros((L, Bn, H), src_emb.dtype)
    c0 = jnp.zeros((L, Bn, H), src_emb.dtype)

    def step(carry, x_t):
        h, c = carry
        x = x_t
        hs, cs = [], []
        for l in range(L):
            g = x @ w_ih[l].T + h[l] @ w_hh[l].T + b_ih[l] + b_hh[l]
            i, f, gg, o = jnp.split(g, 4, axis=-1)
            c_new = jax.nn.sigmoid(f) * c[l] + jax.nn.sigmoid(i) * jnp.tanh(gg)
            h_new = jax.nn.sigmoid(o) * jnp.tanh(c_new)
            hs.append(h_new); cs.append(c_new)
            x = h_new
        hid = x
        scores = jnp.einsum('sbh,bh->bs', Wh_s, hid)
        scores = jnp.where(mask_src, scores, NEG)
        almt = jax.nn.softmax(scores, axis=-1)
        ctx = jnp.einsum('bs,sbd->bd', almt, src_outputs)
        hid_cat = jnp.concatenate([hid, ctx], axis=-1) @ W_hid.T + b_hid
        ctx_emb = jnp.einsum('bs,sbe->be', almt, src_emb)
        hid_res = _nc_residual((ctx_emb, hid_cat), RATIOS)
        log_prob = jax.nn.log_softmax(hid_res @ emb_table.T, axis=-1)
        return (jnp.stack(hs), jnp.stack(cs)), (log_prob, almt)

    _, (log_probs, almts) = jax.lax.scan(step, (h0, c0), x_all)
    return log_probs, almts  # [T, BL, V], [T, BL, S]


_pmapped = None


def _get_pmapped():
    global _pmapped
    if _pmapped is None:
        devs = jax.devices()[:NCORES]
        _pmapped = jax.pmap(
            _decode_local,
            in_axes=(0, 0, 0, 0, 0, None, None, None, None, None, None, None, None),
            devices=devs,
        )
    return _pmapped


def kernel(sot_id, src_emb, src_outputs, mask_src, target,
           emb_table, w_ih, w_hh, b_ih, b_hh, Wa, W_hid, b_hid):
    src_emb = np.asarray(src_emb, np.float32)
    src_outputs = np.asarray(src_outputs, np.float32)
    mask_src = np.asarray(mask_src, bool)
    target_np = np.asarray(target)
    emb_table = np.asarray(emb_table, np.float32)
    w_ih = np.asarray(w_ih, np.float32)
    w_hh = np.asarray(w_hh, np.float32)
    b_ih = np.asarray(b_ih, np.float32)
    b_hh = np.asarray(b_hh, np.float32)
    Wa = np.asarray(Wa, np.float32)
    W_hid = np.asarray(W_hid, np.float32)
    b_hid = np.asarray(b_hid, np.float32)

    # Teacher forcing: input token at step 0 is sot_id, at step t is target[t-1].
    # Pre-gather the input embeddings on host (pure layout prep).
    inp_tokens = np.empty((T, B), np.int64)
    inp_tokens[0, :] = int(sot_id)
    inp_tokens[1:, :] = target_np[:-1, :]
    x_all = emb_table[inp_tokens]  # [T, B, E]

    # Shard batch across 8 cores (axis 1 of [S,B,*] / [T,B] tensors).
    se_sh = np.ascontiguousarray(
        src_emb.reshape(S, NCORES, BL, E).transpose(1, 0, 2, 3))
    so_sh = np.ascontiguousarray(
        src_outputs.reshape(S, NCORES, BL, H).transpose(1, 0, 2, 3))
    mask_sh = np.ascontiguousarray(mask_src.reshape(NCORES, BL, S))
    tgt_sh = np.ascontiguousarray(
        target_np.astype(np.int32).reshape(T, NCORES, BL).transpose(1, 0, 2))
    x_sh = np.ascontiguousarray(
        x_all.reshape(T, NCORES, BL, E).transpose(1, 0, 2, 3))

    f = _get_pmapped()
    lp_sh, al_sh = f(se_sh, so_sh, mask_sh, tgt_sh, x_sh,
                     w_ih, w_hh, b_ih, b_hh, Wa, W_hid, b_hid, emb_table)
    lp_sh = np.asarray(lp_sh)  # [8, T, BL, V]
    al_sh = np.asarray(al_sh)  # [8, T, BL, S]
    log_probs = np.ascontiguousarray(
        lp_sh.transpose(1, 0, 2, 3).reshape(T, B, V))
    almts = np.ascontiguousarray(
        al_sh.transpose(1, 0, 2, 3).reshape(T, B, S))
    return log_probs, almts
